# revision 1
# baseline (speedup 1.0000x reference)
"""ConvGAU (gated attention unit with 1x1 conv projections) on 8 TRN2 NeuronCores.

Data-parallel: B=16 images sharded 2-per-core across 8 cores, no cross-core
communication.

v3: bf16 projections/sim/out-projection + fp8(e4m3) attention-V matmuls.
Measured on HW: bf16 and fp8 matmuls run ~2x the cost-model rate (fp8 non-DR
~0.13 ns/row with rotating stationary — faster than DoubleRow, whose weight
loads don't pipeline). Per-image compute (C=256, N=48*48=2304, HID=512, QK=96):

  q,k  = silu(w_qk x)            one tile [96, 2, N] bf16 (dim1: 0=q, 1=k)
  vT_j = silu(x_j^T w_v)         fp8 pair tiles [128, 2, 512] (j-pairs)
  gate = silu(w_g x)             bf16 pair tiles [128, 2, N] (hs-pairs)
  sim  = k_j^T q_chunk           bf16 matmuls into 2-bank PSUM pairs
  AT   = relu(sim)^2 -> fp8      relu PSUM->SBUF bf16 (DVE/ACT) + square
                                 SBUF->fp8 (Pool/ACT/DVE), paired ops
  V    += vT^T @ AT              fp8 matmuls, PSUM pair accumulation
  Vg   = V * gate                DVE pair multiplies -> bf16
  out  = w_o^T Vg + x            bf16 matmuls; residual pair STT; DMA out

Cross-image software pipelining: image i's V/gating/out-projection (PE-heavy)
emitted before image i+1's projections+sims each chunk step, so at-tile tag
reuse follows PE program order. Gate projections are deferred out of the
pipeline-fill head to balance ACT.
"""

import numpy as np
import ml_dtypes
from contextlib import ExitStack

import concourse.bass as bass
import concourse.tile as tile
from concourse import bacc
from concourse import mybir
from concourse.bass_utils import run_bass_kernel_spmd

B, C, N = 16, 256, 48 * 48
HID, QK = 512, 96
NCORES = 8
BPC = B // NCORES

F32 = mybir.dt.float32
BF = mybir.dt.bfloat16
F8 = mybir.dt.float8e4
AF = mybir.ActivationFunctionType
ALU = mybir.AluOpType
E4 = ml_dtypes.float8_e4m3
BF16 = ml_dtypes.bfloat16

NCH = [(0, 512), (512, 512), (1024, 512), (1536, 512), (2048, 256)]
PAIRS = [[0, 1], [2, 3], [4, 5], [6, 7], [8]]  # j-pairs completed by chunk c
NJP = 9
# relu^2 is two passes (hardware: one PSUM operand max per DVE op, Pool can't
# read PSUM): relu PSUM->SBUF bf16 on DVE('D')/ACT('A'), square SBUF->fp8 on
# Pool('P')/ACT('A')/DVE('D'). Patterns balance engine busy-time.
RELU_PAT = "AD"
SQ_PAT = "PPAPPDPPD"


def build_bass(with_bias: bool = False, reps: int = 1) -> bass.Bass:
    nc = bacc.Bacc("TRN2", target_bir_lowering=False, debug=False)

    xb_d = nc.dram_tensor("x_bf", [BPC, 128, 2, N], BF, kind="ExternalInput").ap()
    xf_d = nc.dram_tensor("x_f32", [BPC, C, N], F32, kind="ExternalInput").ap()
    wqk_d = nc.dram_tensor("w_qk_bf", [128, 2, 2 * QK], BF, kind="ExternalInput").ap()
    wvg_d = nc.dram_tensor("w_vg_bf", [128, 2, 2 * HID], BF, kind="ExternalInput").ap()
    wo_d = nc.dram_tensor("w_o_bf", [128, 4, C], BF, kind="ExternalInput").ap()
    bp_d = bv_d = None
    if with_bias:
        bp_d = nc.dram_tensor("b_pack", [128, 8], F32, kind="ExternalInput").ap()
        bv_d = nc.dram_tensor("b_v_bc", [128, HID], F32, kind="ExternalInput").ap()
    out_d = nc.dram_tensor("out", [BPC, C, N], F32, kind="ExternalOutput").ap()

    xf_re = [xf_d[i].rearrange("(two p) n -> p two n", two=2) for i in range(BPC)]
    out_re = [out_d[i].rearrange("(two p) n -> p two n", two=2) for i in range(BPC)]

    with tile.TileContext(nc) as tc, ExitStack() as ctx:
        consts = ctx.enter_context(tc.tile_pool(name="consts", bufs=1))
        xbp = ctx.enter_context(tc.tile_pool(name="xbp", bufs=2))
        qkp = ctx.enter_context(tc.tile_pool(name="qkp", bufs=1))
        vtp = ctx.enter_context(tc.tile_pool(name="vtp", bufs=2))
        atp = ctx.enter_context(tc.tile_pool(name="atp", bufs=1))
        rlp = ctx.enter_context(tc.tile_pool(name="rlp", bufs=4))
        gp = ctx.enter_context(tc.tile_pool(name="gp", bufs=2))
        vgp = ctx.enter_context(tc.tile_pool(name="vgp", bufs=2))
        xrp = ctx.enter_context(tc.tile_pool(name="xrp", bufs=3))
        obp = ctx.enter_context(tc.tile_pool(name="obp", bufs=2))
        ppp = ctx.enter_context(tc.tile_pool(name="ppp", bufs=2, space="PSUM"))
        vpp = ctx.enter_context(tc.tile_pool(name="vpp", bufs=2, space="PSUM"))

        wqk_sb = consts.tile([128, 2, 2 * QK], BF, name="wqk_sb", tag="wqk")
        wvg_sb = consts.tile([128, 2, 2 * HID], BF, name="wvg_sb", tag="wvg")
        wo_sb = consts.tile([128, 4, C], BF, name="wo_sb", tag="wo")
        nc.gpsimd.dma_start(wqk_sb[:], wqk_d[:])
        nc.gpsimd.dma_start(wvg_sb[:], wvg_d[:])
        nc.gpsimd.dma_start(wo_sb[:], wo_d[:])
        bp_sb = bv_sb = None
        if with_bias:
            bp_sb = consts.tile([128, 8], F32, name="bp_sb", tag="bp")
            bv_sb = consts.tile([128, HID], F32, name="bv_sb", tag="bv")
            nc.gpsimd.dma_start(bp_sb[:], bp_d[:])
            nc.gpsimd.dma_start(bv_sb[:], bv_d[:])

        # per-image-parity live tiles
        xb_t = [None, None]
        qk_t = [None, None]
        vt_t = [{}, {}]
        at_t = [{}, {}]
        g_t = [{}, {}]
        pending = []
        route_cnt = [0]

        def prefetch_x(img, c):
            par = img % 2
            n0, S = NCH[c]
            if c == 0:
                xb_t[par] = xbp.tile([128, 2, N], BF, name=f"xb{par}", tag="xb")
            nc.sync.dma_start(xb_t[par][:, :, n0:n0 + S],
                              xb_d[img, :, :, n0:n0 + S])

        def s1_chunk(img, c):
            par = img % 2
            n0, S = NCH[c]
            if c == 0:
                qk_t[par] = qkp.tile([QK, 2, N], BF, name=f"qk{par}", tag="qk")
            xb = xb_t[par]

            # q and k projections into one psum pair (q bank0, k bank1)
            ps = ppp.tile([128, 2, 512], F32, name="ps_qk", tag="pp")
            for i, woff in ((0, 0), (1, QK)):
                for ci in range(2):
                    nc.tensor.matmul(ps[0:QK, i, 0:S],
                                     wqk_sb[:, ci, woff:woff + QK],
                                     xb[:, ci, n0:n0 + S],
                                     start=(ci == 0), stop=(ci == 1))
            if not with_bias:
                nc.scalar.activation(qk_t[par][:, :, n0:n0 + S],
                                     ps[0:QK, :, 0:S], AF.Silu)
            else:
                for i in range(2):
                    nc.scalar.activation(qk_t[par][:, i, n0:n0 + S],
                                         ps[0:QK, i, 0:S], AF.Silu,
                                         bias=bp_sb[0:QK, i:i + 1])

            # vT for this chunk's j-pairs -> fp8 pair tiles
            for jp in PAIRS[c]:
                ps = ppp.tile([128, 2, 512], F32, name="ps_v", tag="pp")
                for i in range(2):
                    j = 2 * jp + i
                    for ci in range(2):
                        nc.tensor.matmul(ps[:, i, :],
                                         xb[:, ci, j * 128:(j + 1) * 128],
                                         wvg_sb[:, ci, 0:HID],
                                         start=(ci == 0), stop=(ci == 1))
                if with_bias:
                    for i in range(2):
                        nc.vector.tensor_add(ps[:, i, :], ps[:, i, :], bv_sb[:])
                vt = vtp.tile([128, 2, 512], F8, name="vt", tag=f"vt{jp}")
                nc.scalar.activation(vt[:], ps[:], AF.Silu)
                vt_t[par][jp] = vt

            # sims newly enabled by this chunk: the new k-pairs against all
            # q-chunks <= c (needed earliest downstream, emitted first), then
            # this chunk's q against all earlier k-pairs
            new = ([(c2, jp) for jp in PAIRS[c] for c2 in range(c + 1)] +
                   [(c, jp) for c2 in range(c) for jp in PAIRS[c2]])
            for (nc_, jp) in new:
                emit_sim(par, nc_, jp)

        def emit_sim(par, nc_, jp):
            m0, Sm = NCH[nc_]
            qk = qk_t[par]
            ps = ppp.tile([128, 2, 512], F32, name="ps_sim", tag="pp")
            for i in range(2):
                j = 2 * jp + i
                nc.tensor.matmul(ps[:, i, 0:Sm],
                                 qk[:, 1, j * 128:(j + 1) * 128],
                                 qk[:, 0, m0:m0 + Sm],
                                 start=True, stop=True)
            cnt = route_cnt[0]
            rl = rlp.tile([128, 2, 512], BF, name="rl", tag="rl")
            if RELU_PAT[cnt % len(RELU_PAT)] == "D":
                nc.vector.tensor_scalar_max(rl[:, :, 0:Sm], ps[:, :, 0:Sm], 0.0)
            else:
                nc.scalar.activation(rl[:, :, 0:Sm], ps[:, :, 0:Sm], AF.Relu)
            at = atp.tile([128, 2, Sm], F8, name="at", tag=f"at{nc_}_{jp}")
            sq = SQ_PAT[cnt % len(SQ_PAT)]
            if sq == "P":
                nc.gpsimd.tensor_mul(at[:], rl[:, :, 0:Sm], rl[:, :, 0:Sm])
            elif sq == "A":
                nc.scalar.activation(at[:], rl[:, :, 0:Sm], AF.Square)
            else:
                nc.vector.tensor_mul(at[:], rl[:, :, 0:Sm], rl[:, :, 0:Sm])
            route_cnt[0] += 1
            at_t[par][(nc_, jp)] = at

        def gate_chunk(img, c):
            par = img % 2
            n0, S = NCH[c]
            xb = xb_t[par]
            if c == 0:
                for hp in range(2):
                    g_t[par][hp] = gp.tile([128, 2, N], BF, name=f"g{hp}",
                                           tag=f"g{hp}")
            for hp in range(2):
                ps = ppp.tile([128, 2, 512], F32, name="ps_g", tag="pp")
                for i in range(2):
                    hs = 2 * hp + i
                    for ci in range(2):
                        nc.tensor.matmul(
                            ps[:, i, 0:S],
                            wvg_sb[:, ci, HID + hs * 128:HID + (hs + 1) * 128],
                            xb[:, ci, n0:n0 + S],
                            start=(ci == 0), stop=(ci == 1))
                if not with_bias:
                    nc.scalar.activation(g_t[par][hp][:, :, n0:n0 + S],
                                         ps[:, :, 0:S], AF.Silu)
                else:
                    for i in range(2):
                        hs = 2 * hp + i
                        nc.scalar.activation(g_t[par][hp][:, i, n0:n0 + S],
                                             ps[:, i, 0:S], AF.Silu,
                                             bias=bp_sb[:, 2 + hs:3 + hs])

        def flush_pending():
            if not pending:
                return
            img, c, vgs, xr = pending.pop()
            n0, S = NCH[c]
            ps = ppp.tile([128, 2, 512], F32, name="ps_o", tag="pp")
            for os_ in range(2):
                for hs in range(4):
                    nc.tensor.matmul(ps[:, os_, 0:S],
                                     wo_sb[:, hs, os_ * 128:(os_ + 1) * 128],
                                     vgs[hs // 2][:, hs % 2, 0:S],
                                     start=(hs == 0), stop=(hs == 3),
                                     skip_group_check=True)
            ob = obp.tile([128, 2, 512], F32, name="ob", tag="ob")
            if not with_bias:
                nc.vector.scalar_tensor_tensor(ob[:, :, 0:S], ps[:, :, 0:S],
                                               0.0, xr[:, :, 0:S],
                                               ALU.add, ALU.add)
            else:
                for os_ in range(2):
                    nc.vector.scalar_tensor_tensor(
                        ob[:, os_, 0:S], ps[:, os_, 0:S],
                        bp_sb[:, 6 + os_:7 + os_], xr[:, os_, 0:S],
                        ALU.add, ALU.add)
            nc.sync.dma_start(out_re[img][:, :, n0:n0 + S], ob[:, :, 0:S])

        def s2_chunk(img, c):
            par = img % 2
            n0, S = NCH[c]
            # residual stream for this chunk (consumed at the next flush)
            xr = xrp.tile([128, 2, 512], F32, name="xr", tag="xr")
            nc.sync.dma_start(xr[:, :, 0:S], xf_re[img][:, :, n0:n0 + S])

            vps = [vpp.tile([128, 2, 512], F32, name=f"vps{hp}", tag="vp")
                   for hp in range(2)]
            first = True
            for jp in range(NJP):
                at = at_t[par][(c, jp)]
                vt = vt_t[par][jp]
                for i in range(2):
                    for hs in range(4):
                        nc.tensor.matmul(vps[hs // 2][:, hs % 2, 0:S],
                                         vt[:, i, hs * 128:(hs + 1) * 128],
                                         at[:, i, 0:S],
                                         start=(jp == 0 and i == 0),
                                         stop=(jp == NJP - 1 and i == 1),
                                         skip_group_check=True)
                if first:
                    # out-projection of the previous chunk, emitted here so
                    # the PE fills the gating-latency window
                    flush_pending()
                    first = False
            vgs = []
            for hp in range(2):
                vg = vgp.tile([128, 2, 512], BF, name="vg", tag=f"vg{hp}")
                nc.vector.tensor_mul(vg[:, :, 0:S], vps[hp][:, :, 0:S],
                                     g_t[par][hp][:, :, n0:n0 + S])
                vgs.append(vg)
            pending.append((img, c, vgs, xr))

        for rep in range(reps):
            img0, img1 = 0, 1
            for c in range(len(NCH)):
                prefetch_x(img0, c)
                s1_chunk(img0, c)
            # middle: image-0 consumption (PE-heavy) emitted BEFORE image-1
            # production so at-tile tag reuse follows PE program order
            for c in range(len(NCH)):
                prefetch_x(img1, c)
                gate_chunk(img0, c)
                s2_chunk(img0, c)
                s1_chunk(img1, c)
            for c in range(len(NCH)):
                gate_chunk(img1, c)
                s2_chunk(img1, c)
            flush_pending()
    nc.compile()
    return nc


_CACHE = {}


def _get_nc(with_bias: bool) -> bass.Bass:
    if with_bias not in _CACHE:
        _CACHE[with_bias] = build_bass(with_bias)
    return _CACHE[with_bias]


def _make_in_maps(inputs: dict):
    x = np.ascontiguousarray(np.asarray(inputs["x"], dtype=np.float32))
    w_hidden = np.asarray(inputs["w_hidden"], dtype=np.float32)
    b_hidden = np.asarray(inputs["b_hidden"], dtype=np.float32)
    w_qk = np.asarray(inputs["w_qk"], dtype=np.float32)
    b_qk = np.asarray(inputs["b_qk"], dtype=np.float32)
    w_out = np.asarray(inputs["w_out"], dtype=np.float32)
    b_out = np.asarray(inputs["b_out"], dtype=np.float32)

    with_bias = bool(np.any(b_hidden != 0.0) or np.any(b_qk != 0.0)
                     or np.any(b_out != 0.0))

    xs = x.reshape(B, C, N)
    xb = np.ascontiguousarray(
        xs.reshape(B, 2, 128, N).transpose(0, 2, 1, 3)).astype(BF16)
    wqk_bf = np.ascontiguousarray(
        w_qk.T.reshape(2, 128, 2 * QK).transpose(1, 0, 2)).astype(BF16)
    wvg_bf = np.ascontiguousarray(
        w_hidden.T.reshape(2, 128, 2 * HID).transpose(1, 0, 2)).astype(BF16)
    wo_bf = np.ascontiguousarray(
        w_out.T.reshape(4, 128, C).transpose(1, 0, 2)).astype(BF16)

    base = {"w_qk_bf": wqk_bf, "w_vg_bf": wvg_bf, "w_o_bf": wo_bf}
    if with_bias:
        b_pack = np.zeros((128, 8), np.float32)
        b_pack[:QK, 0] = b_qk[:QK]
        b_pack[:QK, 1] = b_qk[QK:]
        b_pack[:, 2:6] = b_hidden[HID:].reshape(4, 128).T
        b_pack[:, 6:8] = b_out.reshape(2, 128).T
        base["b_pack"] = b_pack
        base["b_v_bc"] = np.ascontiguousarray(
            np.tile(b_hidden[None, :HID], (128, 1)))

    in_maps = [
        {**base,
         "x_bf": np.ascontiguousarray(xb[i * BPC:(i + 1) * BPC]),
         "x_f32": np.ascontiguousarray(xs[i * BPC:(i + 1) * BPC])}
        for i in range(NCORES)
    ]
    return in_maps, with_bias


def _run(inputs: dict, trace: bool = False):
    in_maps, with_bias = _make_in_maps(inputs)
    nc = _get_nc(with_bias)
    res = run_bass_kernel_spmd(nc, in_maps, core_ids=list(range(NCORES)),
                               trace=trace)
    out = np.concatenate([res.results[i]["out"] for i in range(NCORES)], axis=0)
    return out.reshape(B, C, 48, 48), res


def kernel(**inputs) -> np.ndarray:
    out, _ = _run(inputs, trace=False)
    return out



# revision 2
# speedup vs baseline: 1.1334x; 1.1334x over previous
"""ConvGAU (gated attention unit with 1x1 conv projections) on 8 TRN2 NeuronCores.

Data-parallel: B=16 images sharded 2-per-core across 8 cores, no cross-core
communication.

v3: bf16 projections/sim/out-projection + fp8(e4m3) attention-V matmuls.
Measured on HW: bf16 and fp8 matmuls run ~2x the cost-model rate (fp8 non-DR
~0.13 ns/row with rotating stationary — faster than DoubleRow, whose weight
loads don't pipeline). Per-image compute (C=256, N=48*48=2304, HID=512, QK=96):

  q,k  = silu(w_qk x)            one tile [96, 2, N] bf16 (dim1: 0=q, 1=k)
  vT_j = silu(x_j^T w_v)         fp8 pair tiles [128, 2, 512] (j-pairs)
  gate = silu(w_g x)             bf16 pair tiles [128, 2, N] (hs-pairs)
  sim  = k_j^T q_chunk           bf16 matmuls into 2-bank PSUM pairs
  AT   = relu(sim)^2 -> fp8      relu PSUM->SBUF bf16 (DVE/ACT) + square
                                 SBUF->fp8 (Pool/ACT/DVE), paired ops
  V    += vT^T @ AT              fp8 matmuls, PSUM pair accumulation
  Vg   = V * gate                DVE pair multiplies -> bf16
  out  = w_o^T Vg + x            bf16 matmuls; residual pair STT; DMA out

Cross-image software pipelining: image i's V/gating/out-projection (PE-heavy)
emitted before image i+1's projections+sims each chunk step, so at-tile tag
reuse follows PE program order. Gate projections are deferred out of the
pipeline-fill head to balance ACT.
"""

import numpy as np
import ml_dtypes
from contextlib import ExitStack

import concourse.bass as bass
import concourse.tile as tile
from concourse import bacc
from concourse import mybir
from concourse.bass_utils import run_bass_kernel_spmd

B, C, N = 16, 256, 48 * 48
HID, QK = 512, 96
NCORES = 8
BPC = B // NCORES

F32 = mybir.dt.float32
BF = mybir.dt.bfloat16
F8 = mybir.dt.float8e4
AF = mybir.ActivationFunctionType
ALU = mybir.AluOpType
E4 = ml_dtypes.float8_e4m3
BF16 = ml_dtypes.bfloat16

NCH = [(0, 512), (512, 512), (1024, 512), (1536, 512), (2048, 256)]
PAIRS = [[0, 1], [2, 3], [4, 5], [6, 7], [8]]  # j-pairs completed by chunk c
NJP = 9
# relu^2 is two passes (hardware: one PSUM operand max per DVE op, Pool can't
# read PSUM): relu PSUM->SBUF bf16 on DVE('D')/ACT('A'), square SBUF->fp8 on
# Pool('P')/ACT('A')/DVE('D'). Patterns balance engine busy-time.
RELU_PAT = "AD"
SQ_PAT = "PPAPPDPPD"


def build_bass(with_bias: bool = False, reps: int = 1) -> bass.Bass:
    nc = bacc.Bacc("TRN2", target_bir_lowering=False, debug=False)

    xb_d = nc.dram_tensor("x_bf", [BPC, 128, 2, N], BF, kind="ExternalInput").ap()
    xf_d = nc.dram_tensor("x_f32", [BPC, C, N], F32, kind="ExternalInput").ap()
    wqk_d = nc.dram_tensor("w_qk_bf", [128, 2, 2 * QK], BF, kind="ExternalInput").ap()
    wvg_d = nc.dram_tensor("w_vg_bf", [128, 2, 2 * HID], BF, kind="ExternalInput").ap()
    wo_d = nc.dram_tensor("w_o_bf", [128, 4, C], BF, kind="ExternalInput").ap()
    bp_d = bv_d = None
    if with_bias:
        bp_d = nc.dram_tensor("b_pack", [128, 8], F32, kind="ExternalInput").ap()
        bv_d = nc.dram_tensor("b_v_bc", [128, HID], F32, kind="ExternalInput").ap()
    out_d = nc.dram_tensor("out", [BPC, C, N], F32, kind="ExternalOutput").ap()

    xf_re = [xf_d[i].rearrange("(two p) n -> p two n", two=2) for i in range(BPC)]
    out_re = [out_d[i].rearrange("(two p) n -> p two n", two=2) for i in range(BPC)]

    with tile.TileContext(nc) as tc, ExitStack() as ctx:
        consts = ctx.enter_context(tc.tile_pool(name="consts", bufs=1))
        xbp = ctx.enter_context(tc.tile_pool(name="xbp", bufs=2))
        qkp = ctx.enter_context(tc.tile_pool(name="qkp", bufs=1))
        vtp = ctx.enter_context(tc.tile_pool(name="vtp", bufs=2))
        atp = ctx.enter_context(tc.tile_pool(name="atp", bufs=1))
        rlp = ctx.enter_context(tc.tile_pool(name="rlp", bufs=4))
        gp = ctx.enter_context(tc.tile_pool(name="gp", bufs=2))
        vgp = ctx.enter_context(tc.tile_pool(name="vgp", bufs=2))
        xrp = ctx.enter_context(tc.tile_pool(name="xrp", bufs=3))
        obp = ctx.enter_context(tc.tile_pool(name="obp", bufs=2))
        ppp = ctx.enter_context(tc.tile_pool(name="ppp", bufs=2, space="PSUM"))
        vpp = ctx.enter_context(tc.tile_pool(name="vpp", bufs=2, space="PSUM"))

        wqk_sb = consts.tile([128, 2, 2 * QK], BF, name="wqk_sb", tag="wqk")
        wvg_sb = consts.tile([128, 2, 2 * HID], BF, name="wvg_sb", tag="wvg")
        wo_sb = consts.tile([128, 4, C], BF, name="wo_sb", tag="wo")
        nc.gpsimd.dma_start(wqk_sb[:], wqk_d[:])
        nc.gpsimd.dma_start(wvg_sb[:], wvg_d[:])
        nc.gpsimd.dma_start(wo_sb[:], wo_d[:])
        bp_sb = bv_sb = None
        if with_bias:
            bp_sb = consts.tile([128, 8], F32, name="bp_sb", tag="bp")
            bv_sb = consts.tile([128, HID], F32, name="bv_sb", tag="bv")
            nc.gpsimd.dma_start(bp_sb[:], bp_d[:])
            nc.gpsimd.dma_start(bv_sb[:], bv_d[:])

        # per-image-parity live tiles
        xb_t = [None, None]
        qk_t = [None, None]
        vt_t = [{}, {}]
        at_t = [{}, {}]
        g_t = [{}, {}]
        pending = []
        route_cnt = [0]

        def prefetch_x(img, c):
            par = img % 2
            n0, S = NCH[c]
            if c == 0:
                xb_t[par] = xbp.tile([128, 2, N], BF, name=f"xb{par}", tag="xb")
            nc.sync.dma_start(xb_t[par][:, :, n0:n0 + S],
                              xb_d[img, :, :, n0:n0 + S])

        def s1_chunk(img, c):
            par = img % 2
            n0, S = NCH[c]
            if c == 0:
                qk_t[par] = qkp.tile([QK, 2, N], BF, name=f"qk{par}", tag="qk")
            xb = xb_t[par]

            # q and k projections into one psum pair (q bank0, k bank1)
            ps = ppp.tile([128, 2, 512], F32, name="ps_qk", tag="pp")
            for i, woff in ((0, 0), (1, QK)):
                for ci in range(2):
                    nc.tensor.matmul(ps[0:QK, i, 0:S],
                                     wqk_sb[:, ci, woff:woff + QK],
                                     xb[:, ci, n0:n0 + S],
                                     start=(ci == 0), stop=(ci == 1))
            if not with_bias:
                nc.scalar.activation(qk_t[par][:, :, n0:n0 + S],
                                     ps[0:QK, :, 0:S], AF.Silu)
            else:
                for i in range(2):
                    nc.scalar.activation(qk_t[par][:, i, n0:n0 + S],
                                         ps[0:QK, i, 0:S], AF.Silu,
                                         bias=bp_sb[0:QK, i:i + 1])

            # vT for this chunk's j-pairs -> fp8 pair tiles
            for jp in PAIRS[c]:
                ps = ppp.tile([128, 2, 512], F32, name="ps_v", tag="pp")
                for i in range(2):
                    j = 2 * jp + i
                    for ci in range(2):
                        nc.tensor.matmul(ps[:, i, :],
                                         xb[:, ci, j * 128:(j + 1) * 128],
                                         wvg_sb[:, ci, 0:HID],
                                         start=(ci == 0), stop=(ci == 1))
                if with_bias:
                    for i in range(2):
                        nc.vector.tensor_add(ps[:, i, :], ps[:, i, :], bv_sb[:])
                vt = vtp.tile([128, 2, 512], F8, name="vt", tag=f"vt{jp}")
                nc.scalar.activation(vt[:], ps[:], AF.Silu)
                vt_t[par][jp] = vt

            # sims newly enabled by this chunk: the new k-pairs against all
            # q-chunks <= c (needed earliest downstream, emitted first), then
            # this chunk's q against all earlier k-pairs
            new = ([(c2, jp) for jp in PAIRS[c] for c2 in range(c + 1)] +
                   [(c, jp) for c2 in range(c) for jp in PAIRS[c2]])
            for (nc_, jp) in new:
                emit_sim(par, nc_, jp)

        def emit_sim(par, nc_, jp):
            m0, Sm = NCH[nc_]
            qk = qk_t[par]
            ps = ppp.tile([128, 2, 512], F32, name="ps_sim", tag="pp")
            for i in range(2):
                j = 2 * jp + i
                nc.tensor.matmul(ps[:, i, 0:Sm],
                                 qk[:, 1, j * 128:(j + 1) * 128],
                                 qk[:, 0, m0:m0 + Sm],
                                 start=True, stop=True)
            cnt = route_cnt[0]
            rl = rlp.tile([128, 2, 512], BF, name="rl", tag="rl")
            if RELU_PAT[cnt % len(RELU_PAT)] == "D":
                nc.vector.tensor_scalar_max(rl[:, :, 0:Sm], ps[:, :, 0:Sm], 0.0)
            else:
                nc.scalar.activation(rl[:, :, 0:Sm], ps[:, :, 0:Sm], AF.Relu)
            at = atp.tile([128, 2, Sm], F8, name="at", tag=f"at{nc_}_{jp}")
            sq = SQ_PAT[cnt % len(SQ_PAT)]
            if sq == "P":
                nc.gpsimd.tensor_mul(at[:], rl[:, :, 0:Sm], rl[:, :, 0:Sm])
            elif sq == "A":
                nc.scalar.activation(at[:], rl[:, :, 0:Sm], AF.Square)
            else:
                nc.vector.tensor_mul(at[:], rl[:, :, 0:Sm], rl[:, :, 0:Sm])
            route_cnt[0] += 1
            at_t[par][(nc_, jp)] = at

        def gate_chunk(img, c):
            par = img % 2
            n0, S = NCH[c]
            xb = xb_t[par]
            if c == 0:
                for hp in range(2):
                    g_t[par][hp] = gp.tile([128, 2, N], BF, name=f"g{hp}",
                                           tag=f"g{hp}")
            for hp in range(2):
                ps = ppp.tile([128, 2, 512], F32, name="ps_g", tag="pp")
                for i in range(2):
                    hs = 2 * hp + i
                    for ci in range(2):
                        nc.tensor.matmul(
                            ps[:, i, 0:S],
                            wvg_sb[:, ci, HID + hs * 128:HID + (hs + 1) * 128],
                            xb[:, ci, n0:n0 + S],
                            start=(ci == 0), stop=(ci == 1))
                if not with_bias:
                    nc.scalar.activation(g_t[par][hp][:, :, n0:n0 + S],
                                         ps[:, :, 0:S], AF.Silu)
                else:
                    for i in range(2):
                        hs = 2 * hp + i
                        nc.scalar.activation(g_t[par][hp][:, i, n0:n0 + S],
                                             ps[:, i, 0:S], AF.Silu,
                                             bias=bp_sb[:, 2 + hs:3 + hs])

        def flush_pending():
            if not pending:
                return
            img, c, vgs, xr = pending.pop()
            n0, S = NCH[c]
            ps = ppp.tile([128, 2, 512], F32, name="ps_o", tag="pp")
            for os_ in range(2):
                for hs in range(4):
                    nc.tensor.matmul(ps[:, os_, 0:S],
                                     wo_sb[:, hs, os_ * 128:(os_ + 1) * 128],
                                     vgs[hs // 2][:, hs % 2, 0:S],
                                     start=(hs == 0), stop=(hs == 3),
                                     skip_group_check=True)
            ob = obp.tile([128, 2, 512], F32, name="ob", tag="ob")
            if not with_bias:
                nc.vector.scalar_tensor_tensor(ob[:, :, 0:S], ps[:, :, 0:S],
                                               0.0, xr[:, :, 0:S],
                                               ALU.add, ALU.add)
            else:
                for os_ in range(2):
                    nc.vector.scalar_tensor_tensor(
                        ob[:, os_, 0:S], ps[:, os_, 0:S],
                        bp_sb[:, 6 + os_:7 + os_], xr[:, os_, 0:S],
                        ALU.add, ALU.add)
            nc.sync.dma_start(out_re[img][:, :, n0:n0 + S], ob[:, :, 0:S])

        def s2_chunk(img, c):
            par = img % 2
            n0, S = NCH[c]
            # residual stream for this chunk (consumed at the next flush)
            xr = xrp.tile([128, 2, 512], F32, name="xr", tag="xr")
            nc.sync.dma_start(xr[:, :, 0:S], xf_re[img][:, :, n0:n0 + S])

            vps = [vpp.tile([128, 2, 512], F32, name=f"vps{hp}", tag="vp")
                   for hp in range(2)]
            first = True
            for jp in range(NJP):
                at = at_t[par][(c, jp)]
                vt = vt_t[par][jp]
                for hs in range(4):
                    # DoubleRow: contracts both i k-tiles (2 fp8/cell) in one
                    # matmul — vt/at dim1 is exactly the k-tile pair dim.
                    nc.tensor.matmul(vps[hs // 2][:, hs % 2, 0:S],
                                     vt[:, 0:2, hs * 128:(hs + 1) * 128],
                                     at[:, 0:2, 0:S],
                                     perf_mode=mybir.MatmulPerfMode.DoubleRow,
                                     start=(jp == 0),
                                     stop=(jp == NJP - 1),
                                     skip_group_check=True)
                if first:
                    # out-projection of the previous chunk, emitted here so
                    # the PE fills the gating-latency window
                    flush_pending()
                    first = False
            vgs = []
            for hp in range(2):
                vg = vgp.tile([128, 2, 512], BF, name="vg", tag=f"vg{hp}")
                nc.vector.tensor_mul(vg[:, :, 0:S], vps[hp][:, :, 0:S],
                                     g_t[par][hp][:, :, n0:n0 + S])
                vgs.append(vg)
            pending.append((img, c, vgs, xr))

        for rep in range(reps):
            img0, img1 = 0, 1
            for c in range(len(NCH)):
                prefetch_x(img0, c)
                s1_chunk(img0, c)
            # middle: image-0 consumption (PE-heavy) emitted BEFORE image-1
            # production so at-tile tag reuse follows PE program order
            for c in range(len(NCH)):
                prefetch_x(img1, c)
                gate_chunk(img0, c)
                s2_chunk(img0, c)
                s1_chunk(img1, c)
            for c in range(len(NCH)):
                gate_chunk(img1, c)
                s2_chunk(img1, c)
            flush_pending()
    nc.compile()
    return nc


_CACHE = {}


def _get_nc(with_bias: bool) -> bass.Bass:
    if with_bias not in _CACHE:
        _CACHE[with_bias] = build_bass(with_bias)
    return _CACHE[with_bias]


def _make_in_maps(inputs: dict):
    x = np.ascontiguousarray(np.asarray(inputs["x"], dtype=np.float32))
    w_hidden = np.asarray(inputs["w_hidden"], dtype=np.float32)
    b_hidden = np.asarray(inputs["b_hidden"], dtype=np.float32)
    w_qk = np.asarray(inputs["w_qk"], dtype=np.float32)
    b_qk = np.asarray(inputs["b_qk"], dtype=np.float32)
    w_out = np.asarray(inputs["w_out"], dtype=np.float32)
    b_out = np.asarray(inputs["b_out"], dtype=np.float32)

    with_bias = bool(np.any(b_hidden != 0.0) or np.any(b_qk != 0.0)
                     or np.any(b_out != 0.0))

    xs = x.reshape(B, C, N)
    xb = np.ascontiguousarray(
        xs.reshape(B, 2, 128, N).transpose(0, 2, 1, 3)).astype(BF16)
    wqk_bf = np.ascontiguousarray(
        w_qk.T.reshape(2, 128, 2 * QK).transpose(1, 0, 2)).astype(BF16)
    wvg_bf = np.ascontiguousarray(
        w_hidden.T.reshape(2, 128, 2 * HID).transpose(1, 0, 2)).astype(BF16)
    wo_bf = np.ascontiguousarray(
        w_out.T.reshape(4, 128, C).transpose(1, 0, 2)).astype(BF16)

    base = {"w_qk_bf": wqk_bf, "w_vg_bf": wvg_bf, "w_o_bf": wo_bf}
    if with_bias:
        b_pack = np.zeros((128, 8), np.float32)
        b_pack[:QK, 0] = b_qk[:QK]
        b_pack[:QK, 1] = b_qk[QK:]
        b_pack[:, 2:6] = b_hidden[HID:].reshape(4, 128).T
        b_pack[:, 6:8] = b_out.reshape(2, 128).T
        base["b_pack"] = b_pack
        base["b_v_bc"] = np.ascontiguousarray(
            np.tile(b_hidden[None, :HID], (128, 1)))

    in_maps = [
        {**base,
         "x_bf": np.ascontiguousarray(xb[i * BPC:(i + 1) * BPC]),
         "x_f32": np.ascontiguousarray(xs[i * BPC:(i + 1) * BPC])}
        for i in range(NCORES)
    ]
    return in_maps, with_bias


def _run(inputs: dict, trace: bool = False):
    in_maps, with_bias = _make_in_maps(inputs)
    nc = _get_nc(with_bias)
    res = run_bass_kernel_spmd(nc, in_maps, core_ids=list(range(NCORES)),
                               trace=trace)
    out = np.concatenate([res.results[i]["out"] for i in range(NCORES)], axis=0)
    return out.reshape(B, C, 48, 48), res


def kernel(**inputs) -> np.ndarray:
    out, _ = _run(inputs, trace=False)
    return out



# revision 6
# speedup vs baseline: 1.4349x; 1.2660x over previous
"""ConvGAU (gated attention unit with 1x1 conv projections) on 8 TRN2 NeuronCores.

Data-parallel: B=16 images sharded 2-per-core across 8 cores, no cross-core
communication.

v6: bf16 projections/sim + fp8(e4m3) DoubleRow attention-V matmuls.
HW trace: matmuls run ~1 row/cycle @2.4GHz regardless of dtype; DoubleRow
(2 fp8 weights/cell) is the only 2x and vt/at tiles already carry the k-tile
pair in dim1. fp8 projections were tried and REJECTED: each fp8 cast of x or
a projection weight costs 1.3-1.9e-2 rel err alone (budget 2e-2).

Per-image compute (C=256, N=48*48=2304, HID=512, QK=96):

  q,k  = silu(w_qk x)            one tile [96, 2, N] bf16 (dim1: 0=q, 1=k)
  vT_j = silu(x_j^T w_v)         fp8 pair tiles [128, 2, 512] (j-pairs)
  gate = silu(w_g x)             bf16 pair tiles [128, 2, N] (hs-pairs)
  sim  = k_j^T q_chunk           bf16 matmuls into 2-bank PSUM pairs
  AT   = relu(sim)^2 -> fp8      relu PSUM->SBUF bf16 (DVE/ACT) + square
                                 SBUF->fp8 (Pool/ACT/DVE), paired ops
  V    += vT^T @ AT              fp8 DoubleRow matmuls, PSUM pair accumulation
  Vg   = V * gate                DVE pair multiplies -> bf16
  out  = w_o^T Vg + x            bf16 matmuls; residual pair STT; DMA out

Cross-image software pipelining: image i's V/gating/out-projection (PE-heavy)
emitted before image i+1's projections+sims each chunk step, so at-tile tag
reuse follows PE program order. Gate projections are deferred out of the
pipeline-fill head to balance ACT.
"""

import numpy as np
import ml_dtypes
from contextlib import ExitStack

import concourse.bass as bass
import concourse.tile as tile
from concourse import bacc
from concourse import mybir
from concourse.bass_utils import run_bass_kernel_spmd

B, C, N = 16, 256, 48 * 48
HID, QK = 512, 96
NCORES = 8
BPC = B // NCORES

F32 = mybir.dt.float32
BF = mybir.dt.bfloat16
F8 = mybir.dt.float8e4
AF = mybir.ActivationFunctionType
ALU = mybir.AluOpType
DR = mybir.MatmulPerfMode.DoubleRow
E4 = ml_dtypes.float8_e4m3
BF16 = ml_dtypes.bfloat16

WSCALE = 256.0   # host-side weight scale so fp8 weights stay in normal range
INV = 1.0 / WSCALE

NCH = [(0, 512), (512, 512), (1024, 512), (1536, 512), (2048, 256)]
PAIRS = [[0, 1], [2, 3], [4, 5], [6, 7], [8]]  # j-pairs completed by chunk c
NJP = 9
# relu^2 is two passes (hardware: one PSUM operand max per DVE op, Pool can't
# read PSUM): relu PSUM->SBUF bf16 on DVE('D')/ACT('A'), square SBUF->fp8 on
# Pool('P')/ACT('A')/DVE('D'). Patterns balance engine busy-time: HW per-op
# costs are ACT ~1.02us, DVE TT ~1.34us, Pool TT ~1.92us (reps=9 trace showed
# Pool 84.5% busy > PE 81.7% with the old P6/A1/D2 split; ACT idled at 56%).
RELU_PAT = "AD"
SQ_PAT = "APADAPAAPADAPAAPDA"


def build_bass(with_bias: bool = False, reps: int = 1) -> bass.Bass:
    nc = bacc.Bacc("TRN2", target_bir_lowering=False, debug=False)

    xb_d = nc.dram_tensor("x_bf", [BPC, 128, 2, N], BF, kind="ExternalInput").ap()
    xf_d = nc.dram_tensor("x_f32", [BPC, C, N], F32, kind="ExternalInput").ap()
    wqk_d = nc.dram_tensor("w_qk_bf", [128, 2, 2 * QK], BF, kind="ExternalInput").ap()
    wvg_d = nc.dram_tensor("w_vg_bf", [128, 2, 2 * HID], BF, kind="ExternalInput").ap()
    wo_d = nc.dram_tensor("w_o_bf", [128, 4, C], BF, kind="ExternalInput").ap()
    bp_d = bv_d = None
    if with_bias:
        bp_d = nc.dram_tensor("b_pack", [128, 8], F32, kind="ExternalInput").ap()
        bv_d = nc.dram_tensor("b_v_bc", [128, HID], F32, kind="ExternalInput").ap()
    out_d = nc.dram_tensor("out", [BPC, C, N], F32, kind="ExternalOutput").ap()

    xf_re = [xf_d[i].rearrange("(two p) n -> p two n", two=2) for i in range(BPC)]
    out_re = [out_d[i].rearrange("(two p) n -> p two n", two=2) for i in range(BPC)]

    with tile.TileContext(nc) as tc, ExitStack() as ctx:
        consts = ctx.enter_context(tc.tile_pool(name="consts", bufs=1))
        xbp = ctx.enter_context(tc.tile_pool(name="xbp", bufs=2))
        qkp = ctx.enter_context(tc.tile_pool(name="qkp", bufs=1))
        vtp = ctx.enter_context(tc.tile_pool(name="vtp", bufs=2))
        atp = ctx.enter_context(tc.tile_pool(name="atp", bufs=1))
        rlp = ctx.enter_context(tc.tile_pool(name="rlp", bufs=4))
        gp = ctx.enter_context(tc.tile_pool(name="gp", bufs=2))
        vgp = ctx.enter_context(tc.tile_pool(name="vgp", bufs=2))
        xrp = ctx.enter_context(tc.tile_pool(name="xrp", bufs=3))
        obp = ctx.enter_context(tc.tile_pool(name="obp", bufs=2))
        ppp = ctx.enter_context(tc.tile_pool(name="ppp", bufs=2, space="PSUM"))
        vpp = ctx.enter_context(tc.tile_pool(name="vpp", bufs=2, space="PSUM"))

        wqk_sb = consts.tile([128, 2, 2 * QK], BF, name="wqk_sb", tag="wqk")
        wvg_sb = consts.tile([128, 2, 2 * HID], BF, name="wvg_sb", tag="wvg")
        wo_sb = consts.tile([128, 4, C], BF, name="wo_sb", tag="wo")
        nc.gpsimd.dma_start(wqk_sb[:], wqk_d[:])
        nc.gpsimd.dma_start(wvg_sb[:], wvg_d[:])
        nc.gpsimd.dma_start(wo_sb[:], wo_d[:])
        bp_sb = bv_sb = None
        if with_bias:
            bp_sb = consts.tile([128, 8], F32, name="bp_sb", tag="bp")
            bv_sb = consts.tile([128, HID], F32, name="bv_sb", tag="bv")
            nc.gpsimd.dma_start(bp_sb[:], bp_d[:])
            nc.gpsimd.dma_start(bv_sb[:], bv_d[:])

        # per-image-parity live tiles
        xb_t = [None, None]
        qk_t = [None, None]
        vt_t = [{}, {}]
        at_t = [{}, {}]
        g_t = [{}, {}]
        pending = []
        route_cnt = [0]

        def prefetch_x(img, c):
            par = img % 2
            n0, S = NCH[c]
            if c == 0:
                xb_t[par] = xbp.tile([128, 2, N], BF, name=f"xb{par}", tag="xb")
            nc.sync.dma_start(xb_t[par][:, :, n0:n0 + S],
                              xb_d[img, :, :, n0:n0 + S])

        def s1_chunk(img, c):
            par = img % 2
            n0, S = NCH[c]
            if c == 0:
                qk_t[par] = qkp.tile([QK, 2, N], BF, name=f"qk{par}", tag="qk")
            xb = xb_t[par]

            # q and k projections into one psum pair (q bank0, k bank1)
            ps = ppp.tile([128, 2, 512], F32, name="ps_qk", tag="pp")
            for i, woff in ((0, 0), (1, QK)):
                for ci in range(2):
                    nc.tensor.matmul(ps[0:QK, i, 0:S],
                                     wqk_sb[:, ci, woff:woff + QK],
                                     xb[:, ci, n0:n0 + S],
                                     start=(ci == 0), stop=(ci == 1))
            if not with_bias:
                nc.scalar.activation(qk_t[par][:, :, n0:n0 + S],
                                     ps[0:QK, :, 0:S], AF.Silu)
            else:
                for i in range(2):
                    nc.scalar.activation(qk_t[par][:, i, n0:n0 + S],
                                         ps[0:QK, i, 0:S], AF.Silu,
                                         bias=bp_sb[0:QK, i:i + 1])

            # vT for this chunk's j-pairs -> fp8 pair tiles
            for jp in PAIRS[c]:
                ps = ppp.tile([128, 2, 512], F32, name="ps_v", tag="pp")
                for i in range(2):
                    j = 2 * jp + i
                    for ci in range(2):
                        nc.tensor.matmul(ps[:, i, :],
                                         xb[:, ci, j * 128:(j + 1) * 128],
                                         wvg_sb[:, ci, 0:HID],
                                         start=(ci == 0), stop=(ci == 1))
                if with_bias:
                    for i in range(2):
                        nc.vector.tensor_add(ps[:, i, :], ps[:, i, :], bv_sb[:])
                vt = vtp.tile([128, 2, 512], F8, name="vt", tag=f"vt{jp}")
                nc.scalar.activation(vt[:], ps[:], AF.Silu)
                vt_t[par][jp] = vt

            # sims newly enabled by this chunk: the new k-pairs against all
            # q-chunks <= c (needed earliest downstream, emitted first), then
            # this chunk's q against all earlier k-pairs
            new = ([(c2, jp) for jp in PAIRS[c] for c2 in range(c + 1)] +
                   [(c, jp) for c2 in range(c) for jp in PAIRS[c2]])
            for (nc_, jp) in new:
                emit_sim(par, nc_, jp)

        def emit_sim(par, nc_, jp):
            m0, Sm = NCH[nc_]
            qk = qk_t[par]
            ps = ppp.tile([128, 2, 512], F32, name="ps_sim", tag="pp")
            for i in range(2):
                j = 2 * jp + i
                nc.tensor.matmul(ps[:, i, 0:Sm],
                                 qk[:, 1, j * 128:(j + 1) * 128],
                                 qk[:, 0, m0:m0 + Sm],
                                 start=True, stop=True)
            cnt = route_cnt[0]
            rl = rlp.tile([128, 2, 512], BF, name="rl", tag="rl")
            if RELU_PAT[cnt % len(RELU_PAT)] == "D":
                nc.vector.tensor_scalar_max(rl[:, :, 0:Sm], ps[:, :, 0:Sm], 0.0)
            else:
                nc.scalar.activation(rl[:, :, 0:Sm], ps[:, :, 0:Sm], AF.Relu)
            at = atp.tile([128, 2, Sm], F8, name="at", tag=f"at{nc_}_{jp}")
            sq = SQ_PAT[cnt % len(SQ_PAT)]
            if sq == "P":
                nc.gpsimd.tensor_mul(at[:], rl[:, :, 0:Sm], rl[:, :, 0:Sm])
            elif sq == "A":
                nc.scalar.activation(at[:], rl[:, :, 0:Sm], AF.Square)
            else:
                nc.vector.tensor_mul(at[:], rl[:, :, 0:Sm], rl[:, :, 0:Sm])
            route_cnt[0] += 1
            at_t[par][(nc_, jp)] = at

        def gate_chunk(img, c):
            par = img % 2
            n0, S = NCH[c]
            xb = xb_t[par]
            if c == 0:
                for hp in range(2):
                    g_t[par][hp] = gp.tile([128, 2, N], BF, name=f"g{hp}",
                                           tag=f"g{hp}")
            for hp in range(2):
                ps = ppp.tile([128, 2, 512], F32, name="ps_g", tag="pp")
                for i in range(2):
                    hs = 2 * hp + i
                    for ci in range(2):
                        nc.tensor.matmul(
                            ps[:, i, 0:S],
                            wvg_sb[:, ci, HID + hs * 128:HID + (hs + 1) * 128],
                            xb[:, ci, n0:n0 + S],
                            start=(ci == 0), stop=(ci == 1))
                if not with_bias:
                    nc.scalar.activation(g_t[par][hp][:, :, n0:n0 + S],
                                         ps[:, :, 0:S], AF.Silu)
                else:
                    for i in range(2):
                        hs = 2 * hp + i
                        nc.scalar.activation(g_t[par][hp][:, i, n0:n0 + S],
                                             ps[:, i, 0:S], AF.Silu,
                                             bias=bp_sb[:, 2 + hs:3 + hs])

        def flush_pending():
            if not pending:
                return
            img, c, vgs, xr = pending.pop()
            n0, S = NCH[c]
            ps = ppp.tile([128, 2, 512], F32, name="ps_o", tag="pp")
            for os_ in range(2):
                for hs in range(4):
                    nc.tensor.matmul(ps[:, os_, 0:S],
                                     wo_sb[:, hs, os_ * 128:(os_ + 1) * 128],
                                     vgs[hs // 2][:, hs % 2, 0:S],
                                     start=(hs == 0), stop=(hs == 3),
                                     skip_group_check=True)
            ob = obp.tile([128, 2, 512], F32, name="ob", tag="ob")
            if not with_bias:
                nc.vector.scalar_tensor_tensor(ob[:, :, 0:S], ps[:, :, 0:S],
                                               0.0, xr[:, :, 0:S],
                                               ALU.add, ALU.add)
            else:
                for os_ in range(2):
                    nc.vector.scalar_tensor_tensor(
                        ob[:, os_, 0:S], ps[:, os_, 0:S],
                        bp_sb[:, 6 + os_:7 + os_], xr[:, os_, 0:S],
                        ALU.add, ALU.add)
            nc.sync.dma_start(out_re[img][:, :, n0:n0 + S], ob[:, :, 0:S])

        def s2_chunk(img, c):
            par = img % 2
            n0, S = NCH[c]
            # residual stream for this chunk (consumed at the next flush)
            xr = xrp.tile([128, 2, 512], F32, name="xr", tag="xr")
            nc.sync.dma_start(xr[:, :, 0:S], xf_re[img][:, :, n0:n0 + S])

            vps = [vpp.tile([128, 2, 512], F32, name=f"vps{hp}", tag="vp")
                   for hp in range(2)]
            first = True
            for jp in range(NJP):
                at = at_t[par][(c, jp)]
                vt = vt_t[par][jp]
                for hs in range(4):
                    # DoubleRow: contracts both i k-tiles (2 fp8/cell) in one
                    # matmul — vt/at dim1 is exactly the k-tile pair dim.
                    nc.tensor.matmul(vps[hs // 2][:, hs % 2, 0:S],
                                     vt[:, 0:2, hs * 128:(hs + 1) * 128],
                                     at[:, 0:2, 0:S],
                                     perf_mode=DR,
                                     start=(jp == 0),
                                     stop=(jp == NJP - 1),
                                     skip_group_check=True)
                if first:
                    # out-projection of the previous chunk, emitted here so
                    # the PE fills the gating-latency window
                    flush_pending()
                    first = False
            vgs = []
            for hp in range(2):
                vg = vgp.tile([128, 2, 512], BF, name="vg", tag=f"vg{hp}")
                nc.vector.tensor_mul(vg[:, :, 0:S], vps[hp][:, :, 0:S],
                                     g_t[par][hp][:, :, n0:n0 + S])
                vgs.append(vg)
            pending.append((img, c, vgs, xr))

        for rep in range(reps):
            img0, img1 = 0, 1
            for c in range(len(NCH)):
                prefetch_x(img0, c)
                s1_chunk(img0, c)
            # middle: image-0 consumption (PE-heavy) emitted BEFORE image-1
            # production so at-tile tag reuse follows PE program order
            for c in range(len(NCH)):
                prefetch_x(img1, c)
                gate_chunk(img0, c)
                s2_chunk(img0, c)
                s1_chunk(img1, c)
            for c in range(len(NCH)):
                gate_chunk(img1, c)
                s2_chunk(img1, c)
            flush_pending()
    nc.compile()
    return nc


_CACHE = {}


def _get_nc(with_bias: bool) -> bass.Bass:
    if with_bias not in _CACHE:
        _CACHE[with_bias] = build_bass(with_bias)
    return _CACHE[with_bias]


def _make_in_maps(inputs: dict):
    x = np.ascontiguousarray(np.asarray(inputs["x"], dtype=np.float32))
    w_hidden = np.asarray(inputs["w_hidden"], dtype=np.float32)
    b_hidden = np.asarray(inputs["b_hidden"], dtype=np.float32)
    w_qk = np.asarray(inputs["w_qk"], dtype=np.float32)
    b_qk = np.asarray(inputs["b_qk"], dtype=np.float32)
    w_out = np.asarray(inputs["w_out"], dtype=np.float32)
    b_out = np.asarray(inputs["b_out"], dtype=np.float32)

    with_bias = bool(np.any(b_hidden != 0.0) or np.any(b_qk != 0.0)
                     or np.any(b_out != 0.0))

    xs = x.reshape(B, C, N)
    xb = np.ascontiguousarray(
        xs.reshape(B, 2, 128, N).transpose(0, 2, 1, 3)).astype(BF16)
    wqk_bf = np.ascontiguousarray(
        w_qk.T.reshape(2, 128, 2 * QK).transpose(1, 0, 2)).astype(BF16)
    wvg_bf = np.ascontiguousarray(
        w_hidden.T.reshape(2, 128, 2 * HID).transpose(1, 0, 2)).astype(BF16)
    wo_bf = np.ascontiguousarray(
        w_out.T.reshape(4, 128, C).transpose(1, 0, 2)).astype(BF16)

    base = {"w_qk_bf": wqk_bf, "w_vg_bf": wvg_bf, "w_o_bf": wo_bf}
    if with_bias:
        b_pack = np.zeros((128, 8), np.float32)
        b_pack[:QK, 0] = b_qk[:QK]
        b_pack[:QK, 1] = b_qk[QK:]
        b_pack[:, 2:6] = b_hidden[HID:].reshape(4, 128).T
        b_pack[:, 6:8] = b_out.reshape(2, 128).T
        base["b_pack"] = b_pack
        base["b_v_bc"] = np.ascontiguousarray(
            np.tile(b_hidden[None, :HID], (128, 1)))

    in_maps = [
        {**base,
         "x_bf": np.ascontiguousarray(xb[i * BPC:(i + 1) * BPC]),
         "x_f32": np.ascontiguousarray(xs[i * BPC:(i + 1) * BPC])}
        for i in range(NCORES)
    ]
    return in_maps, with_bias


def _run(inputs: dict, trace: bool = False):
    in_maps, with_bias = _make_in_maps(inputs)
    nc = _get_nc(with_bias)
    res = run_bass_kernel_spmd(nc, in_maps, core_ids=list(range(NCORES)),
                               trace=trace)
    out = np.concatenate([res.results[i]["out"] for i in range(NCORES)], axis=0)
    return out.reshape(B, C, 48, 48), res


def kernel(**inputs) -> np.ndarray:
    out, _ = _run(inputs, trace=False)
    return out
